# revision 67
# baseline (speedup 1.0000x reference)
"""GATv2 gene-graph kernel for 8 Trainium2 NeuronCores (Bass/Tile).

Strategy (data-parallel over batch, per the sharding hint):
- Host: shard batch (B=256 -> 32/core), precompute edge structure as static
  one-hot matrices (edge_index is data, known at trace time).
- All PE matmuls run bf16 (inputs converted on host; PSUM accumulates fp32).
- Per-gene input linear: kc-outer PE matmuls; weights arrive as 10 large
  [128, G*C] bf16 DMAs (one per kc); per-gene bias + LeakyReLU(0.01) fused
  into the ACT evacuation (bias is per-partition = per-channel).
- GATv2 attention: deduped (dst,src) pairs; z = x_l[src]+x_r[dst] via static
  one-hot PE matmuls out of the bf16 XLR tile (b_l/b_r added to XLR by DVE
  during the stage-B evacuation, so no per-batch bias matmuls);
  LeakyReLU(0.2) on ACT; att-dot via DVE mul + multi-dim reduce; segment
  softmax without max-subtraction (logits are tiny) using a degree-6 poly
  exp with ln(edge-count) folded in to handle duplicate edges.
- Aggregation: A^T built per-destination with masked one-hot PE matmuls
  (bf16), then dense bf16 PE matmuls over heads straight out of XLR (the
  message features are never written to DRAM).
- Output MLP: bf16 PE matmuls, W1 arrives as one 8.4MB DMA early.
"""
import sys
from contextlib import ExitStack

import numpy as np

sys.path.insert(0, "/opt/trn_rl_repo")

import ml_dtypes  # noqa: E402
import concourse.bass as bass  # noqa: E402
import concourse.tile as tile  # noqa: E402
from concourse import bacc, mybir  # noqa: E402

bf16 = ml_dtypes.bfloat16
F32 = mybir.dt.float32
BF = mybir.dt.bfloat16
AF = mybir.ActivationFunctionType
ALU = mybir.AluOpType

G, B, IN, C, H = 64, 256, 1280, 128, 4
HC = H * C  # 512
KC = IN // 128  # 10
NCORES = 8
BC = B // NCORES  # 32
HID1, HID2 = 512, 128
ZB = 4  # batch elements per z-group (DVE op granularity)


def _prep_edges(edge_index):
    sl = np.arange(G, dtype=np.int64)
    src = np.concatenate([np.asarray(edge_index[0]), sl])
    dst = np.concatenate([np.asarray(edge_index[1]), sl])
    upairs, cnt = np.unique(dst * G + src, return_counts=True)
    pd = (upairs // G).astype(np.int64)
    ps = (upairs % G).astype(np.int64)
    p_real = len(upairs)
    n_chunks = (p_real + 127) // 128
    P = n_chunks * 128
    seg_len = np.bincount(pd, minlength=G)
    seg_off = np.zeros(G, np.int64)
    seg_off[1:] = np.cumsum(seg_len)[:-1]
    cnt720 = np.zeros(P, np.float32)
    cnt720[:p_real] = (cnt.astype(np.float64) / 720.0).astype(np.float32)
    ps_pad = np.zeros(P, np.int64)
    ps_pad[:p_real] = ps
    pd_pad = np.full(P, G - 1, np.int64)
    pd_pad[:p_real] = pd

    OH = np.zeros((n_chunks, 128, 128), bf16)
    for p in range(P):
        ch, k = p // 128, p % 128
        OH[ch, ps_pad[p], k] = 1
        OH[ch, G + pd_pad[p], k] = 1

    # AT-build plan: per destination d, pieces of its (real) segment per chunk,
    # as zero-masked one-hots [128(pair-in-chunk), 64(g)].
    pieces = []  # (d, ch, is_first, is_last)
    oh_seg = []
    for d in range(G):
        o, l = int(seg_off[d]), int(seg_len[d])
        if l == 0:
            continue  # cannot happen (self loops guarantee l>=1)
        ch_lo, ch_hi = o // 128, (o + l - 1) // 128
        plist = []
        for ch in range(ch_lo, ch_hi + 1):
            lo = max(o, ch * 128)
            hi = min(o + l, (ch + 1) * 128)
            m = np.zeros((128, 64), bf16)
            for p in range(lo, hi):
                m[p % 128, ps_pad[p]] = 1
            plist.append((ch, m))
        for i, (ch, m) in enumerate(plist):
            pieces.append((d, ch, i == 0, i == len(plist) - 1))
            oh_seg.append(m)
    oh_seg = np.stack(oh_seg)  # [n_pieces, 128, 64]

    # segment bounds on the padded list (pads live in d=63's tail; their
    # cnt720=0 makes them contribute 0 to the denominator)
    seg_bounds = []
    for d in range(G):
        o, l = int(seg_off[d]), int(seg_len[d])
        if d == G - 1:
            l += P - p_real
        seg_bounds.append((o, l))

    return dict(P=P, n_chunks=n_chunks, cnt720=cnt720, OH=OH,
                oh_seg=oh_seg, pieces=pieces, seg_bounds=seg_bounds)


def _build(E, lrelu_act=True, poly_exp=True):
    P, n_chunks = E["P"], E["n_chunks"]
    pieces = E["pieces"]
    n_pieces = len(pieces)

    nc = bacc.Bacc("TRN2", target_bir_lowering=False, debug=False)

    def din(name, shape, dt=F32):
        return nc.dram_tensor(name, list(shape), dt, kind="ExternalInput").ap()

    peT = din("peT", [KC, 128, G * BC], BF)
    WinK = din("WinK", [KC, 128, G * C], BF)
    binb = din("binb", [128, G * BC], BF)
    onesv = din("onesv", [1, 64])
    Wl = din("Wl", [C, HC], BF)
    Wr = din("Wr", [C, HC], BF)
    blT = din("blT", [G, HC])
    brT = din("brT", [G, HC])
    OHd = din("OH", [n_chunks, 128, 128], BF)
    OHseg = din("OHseg", [n_pieces, 128, 64], BF)
    cntd = din("cnt720", [128, P])
    attrep = din("attrep", [128, ZB * HC], BF)
    gbias = din("gbias", [C, 1])
    identd = din("ident", [128, 128])
    identb = din("identb", [128, 128], BF)
    W1d = din("W1", [G * C, HID1], BF)
    b1v = din("b1v", [1, HID1])
    W2d = din("W2", [HID1, HID2], BF)
    b2v = din("b2v", [1, HID2])
    W3d = din("W3", [HID2, 1])
    outd = nc.dram_tensor("out", [BC, 1], F32, kind="ExternalOutput").ap()

    with tile.TileContext(nc) as tc, ExitStack() as ctx:
        pers = ctx.enter_context(tc.tile_pool(name="pers", bufs=1))

        # persistent data tiles
        xT = pers.tile([128, G, BC], BF, tag="xT")
        # one tile per 4-batch group: tile-granular dependency tracking
        # would otherwise serialize all of stage C behind all of stage B
        XLRg = [pers.tile([128, ZB * HC], BF, tag=f"XLR{g}", name=f"XLR{g}")
                for g in range(BC // ZB)]

        def XLR(b):
            return XLRg[b // ZB], (b % ZB) * HC
        Sv = pers.tile([128, P], F32, tag="Sv")
        expS = pers.tile([128, P], F32, tag="expS")
        Av = expS  # alpha overwrites the exp values in place
        pt_ = pers.tile([128, P], F32, tag="polyt")
        ATs = pers.tile([64, G, 128], BF, tag="ATs")
        M1 = pers.tile([128, BC, G], BF, tag="M1")

        # constants (tiles declared here; DMAs issued after stage A's first
        # weight tiles so they don't delay the critical path)
        ones_t = pers.tile([1, 64], F32, tag="ones")
        binb_t = pers.tile([128, G * BC], BF, tag="binb")
        Wl_t = pers.tile([C, HC], BF, tag="Wl")
        Wr_t = pers.tile([C, HC], BF, tag="Wr")
        blT_t = pers.tile([G, HC], F32, tag="blT")
        brT_t = pers.tile([G, HC], F32, tag="brT")
        lnc_t = pers.tile([128, P], F32, tag="lnc")
        att_t = pers.tile([128, ZB * 4, C], BF, tag="att")
        gb_t = pers.tile([C, 1], F32, tag="gb")
        id_t = pers.tile([128, 128], F32, tag="id")
        id_t_bf = pers.tile([128, 128], BF, tag="idb")
        b1_t = pers.tile([1, HID1], F32, tag="b1")
        b2_t = pers.tile([1, HID2], F32, tag="b2")
        W3_t = pers.tile([HID2, 1], F32, tag="W3")
        oh_t = pers.tile([128, n_chunks, 128], BF, tag="ohall")
        ohs_t = pers.tile([128, n_pieces, 64], BF, tag="ohseg")
        W1_t = pers.tile([128, G, HID1], BF, tag="W1")
        W2_t = pers.tile([128, 4, HID2], BF, tag="W2")

        def emit_consts():
            nc.sync.dma_start(ones_t[:], onesv)
            nc.sync.dma_start(binb_t[:], binb)
            nc.sync.dma_start(Wl_t[:], Wl)
            nc.sync.dma_start(Wr_t[:], Wr)
            nc.sync.dma_start(blT_t[:], blT)
            nc.sync.dma_start(brT_t[:], brT)
            nc.sync.dma_start(att_t[:],
                              attrep.rearrange("p (g c) -> p g c", c=C))
            nc.sync.dma_start(id_t[:], identd)
            nc.sync.dma_start(id_t_bf[:], identb)
            nc.sync.dma_start(oh_t[:], OHd.rearrange("n p q -> p n q"))
            nc.sync.dma_start(lnc_t[:], cntd)
            nc.sync.dma_start(gb_t[:], gbias)
            nc.sync.dma_start(b1_t[:], b1v)
            nc.sync.dma_start(b2_t[:], b2v)
            nc.sync.dma_start(W3_t[:], W3d)

        def emit_lrelu(out_ap, in_ap, alpha, eng=None):
            if lrelu_act:
                nc.scalar.activation(out_ap, in_ap, AF.Lrelu, alpha=alpha)
            else:
                (eng or nc.vector).scalar_tensor_tensor(
                    out_ap, in_ap, alpha, in_ap, ALU.mult, ALU.max)

        # ---- Stage A: per-gene input linear ----
        # kc-outer; all 64 genes accumulate in 4 persistent PSUM banks
        # (bank q holds genes 16q..16q+15 as [128c, 16*32b]). Weights arrive
        # as one [128, G*C] bf16 DMA per kc; per-gene bias + LeakyReLU(0.01)
        # fused into the evacuation.
        with tc.tile_pool(name="pep", bufs=2) as pep, \
             tc.tile_pool(name="wkp", bufs=2) as wkp, \
             tc.tile_pool(name="aps", bufs=1, space="PSUM") as aps:
            xbank = [aps.tile([128, 512], F32, tag=f"xb{q}", name=f"xb{q}")
                     for q in range(4)]
            for kc in range(KC):
                wk = wkp.tile([128, G * C], BF, tag="wk")
                nc.sync.dma_start(wk[:], WinK[kc])
                pt = pep.tile([128, G * BC], BF, tag="pe")
                nc.sync.dma_start(pt[:], peT[kc])
                if kc == 0:
                    emit_consts()
                for g in range(G):
                    nc.tensor.matmul(
                        xbank[g // 16][:, (g % 16) * BC:(g % 16 + 1) * BC],
                        wk[:, g * C:(g + 1) * C], pt[:, g * BC:(g + 1) * BC],
                        start=(kc == 0 and g % 16 == 0),
                        stop=(kc == KC - 1 and g % 16 == 15))
            # bias added into PSUM by DVE (idle in this window), so the
            # evacuation is 4 wide bank-sized lrelus instead of 64 per-gene
            for q in range(4):
                nc.vector.tensor_add(xbank[q][:], xbank[q][:],
                                     binb_t[:, q * 512:(q + 1) * 512])
                emit_lrelu(xT[:, q * 16:(q + 1) * 16, :], xbank[q][:], 0.01)

        # ---- gated prefetch of big late-stage tensors ----
        # A 1-element copy from xT makes these DMAs wait for stage A's end,
        # keeping the DMA engines clear for the critical wk/pt stream; they
        # then overlap stages B..E (the data isn't needed until E/F).
        nc.vector.tensor_copy(ohs_t[0:1, 0:1, 0:1], xT[0:1, 0:1, 0:1])
        nc.vector.tensor_copy(W1_t[0:1, 0:1, 0:1], xT[0:1, 0:1, 0:1])
        nc.sync.dma_start(ohs_t[:], OHseg.rearrange("n p g -> p n g"))
        nc.sync.dma_start(W1_t[:], W1d.rearrange("(d p) h -> p d h", p=128))
        nc.sync.dma_start(W2_t[:], W2d.rearrange("(k p) c -> p k c", p=128))

        # ---- Stage B: x_l / x_r transforms ----
        # Biases b_l/b_r ride into XLR via the DVE evacuation adds.
        with tc.tile_pool(name="bps", bufs=4, space="PSUM") as bps:
            for b in range(BC):
                xsl = xT[:, :, b]  # [128c, 64g]
                psl = bps.tile([64, HC], F32, tag="psl")
                nc.tensor.matmul(psl[:], xsl, Wl_t[:], start=True, stop=True)
                xt, xo = XLR(b)
                nc.vector.tensor_add(xt[0:64, xo:xo + HC], psl[:], blT_t[:])
                psr = bps.tile([64, HC], F32, tag="psr")
                nc.tensor.matmul(psr[:], xsl, Wr_t[:], start=True, stop=True)
                nc.vector.tensor_add(xt[64:128, xo:xo + HC], psr[:], brT_t[:])

        # ---- Stage C: pair features + scores ----
        den_t = pers.tile([128, 64], F32, tag="den")
        rden_t = pers.tile([128, 64], F32, tag="rden")
        with tc.tile_pool(name="zps", bufs=3, space="PSUM") as zps, \
             tc.tile_pool(name="zlp", bufs=3) as zlp, \
             tc.tile_pool(name="sap", bufs=2) as sap, \
             tc.tile_pool(name="tps", bufs=2, space="PSUM") as tps:
            seg_bounds = E["seg_bounds"]
            ends_in_ch = [[] for _ in range(n_chunks)]
            for dd in range(G):
                o, l = seg_bounds[dd]
                ends_in_ch[(o + l - 1) // 128].append(dd)

            def emit_score_tail(sat_, ch_):
                # transpose + poly-exp + this chunk's finished segment sums;
                # emitted one chunk late so the PE's in-order queue never
                # stalls on the DVE reduce feeding the transpose
                tp = tps.tile([128, 128], BF, tag="tp")
                nc.tensor.transpose(tp[:], sat_[:], id_t_bf[:])
                sv = Sv[:, ch_ * 128:(ch_ + 1) * 128]
                nc.scalar.activation(sv, tp[:], AF.Copy)
                if poly_exp:
                    # 720*exp(x) ~= (((((x+6)x+30)x+120)x+360)x+720)x+720
                    # then alpha_unnorm = (t+720) * cnt/720 (cnt=0 => pad)
                    pc_ = pt_[:, ch_ * 128:(ch_ + 1) * 128]
                    ec = expS[:, ch_ * 128:(ch_ + 1) * 128]
                    lc = lnc_t[:, ch_ * 128:(ch_ + 1) * 128]
                    nc.vector.scalar_tensor_tensor(pc_, sv, 6.0, sv,
                                                   ALU.add, ALU.mult)
                    for c in (30.0, 120.0, 360.0, 720.0):
                        nc.vector.scalar_tensor_tensor(pc_, pc_, c, sv,
                                                       ALU.add, ALU.mult)
                    nc.vector.scalar_tensor_tensor(ec, pc_, 720.0, lc,
                                                   ALU.add, ALU.mult)

            pending = None
            for ch in range(n_chunks):
                sat = sap.tile([128, 128], BF, tag="sa")
                for bg in range(BC // ZB):
                    if bg == 5 and pending is not None:
                        # previous chunk's tail, five groups late: the DVE
                        # runs ~1us/group behind the PE, so this is when its
                        # sat is actually complete and the PE's in-order
                        # queue won't stall on the transpose
                        emit_score_tail(*pending)
                        pending = None
                    zt = zlp.tile([128, ZB * 4, C], BF, tag="zt")
                    for half in range(2):
                        # two matmuls into adjacent PSUM banks, one wide
                        # evacuation (halves the ACT instruction count)
                        zp = zps.tile([128, 2 * HC], F32, tag="zp")
                        for bi2 in range(2):
                            b = bg * ZB + half * 2 + bi2
                            xt, xo = XLR(b)
                            nc.tensor.matmul(
                                zp[:, bi2 * HC:(bi2 + 1) * HC],
                                oh_t[:, ch, :],
                                xt[:, xo:xo + HC],
                                start=True, stop=True)
                        emit_lrelu(zt[:, half * 8:(half + 1) * 8, :],
                                   zp[:], 0.2)
                    nc.vector.tensor_mul(zt[:], zt[:], att_t[:])
                    # pairwise in-place folds shrink the (slow, 1x-rate)
                    # reduce read from 2048 to 512 elements per lane
                    nc.vector.tensor_add(zt[:, :, 0:64], zt[:, :, 0:64],
                                         zt[:, :, 64:128])
                    nc.vector.tensor_add(zt[:, :, 0:32], zt[:, :, 0:32],
                                         zt[:, :, 32:64])
                    with nc.allow_low_precision(reason="score sums are tiny"):
                        nc.vector.tensor_reduce(
                            sat[:, bg * ZB * 4:(bg + 1) * ZB * 4],
                            zt[:, :, 0:32],
                            axis=mybir.AxisListType.X, op=ALU.add)
                pending = (sat, ch)
            emit_score_tail(*pending)

        # ---- Stage D: softmax normalization ----
        if not poly_exp:
            nc.scalar.activation(expS[:], Sv[:], AF.Exp)
            nc.vector.tensor_mul(expS[:], expS[:], lnc_t[:])
        for d in range(G):
            o, l = seg_bounds[d]
            nc.vector.tensor_reduce(den_t[:, d:d + 1], expS[:, o:o + l],
                                    axis=mybir.AxisListType.X, op=ALU.add)
        nc.vector.reciprocal(rden_t[:], den_t[:])

        # ---- Stage E: build AT[g, d, bh] (bf16), pipelined with the
        # alpha normalization: each chunk's transpose is emitted as soon
        # as the segments covering its columns are scaled ----
        with tc.tile_pool(name="etp", bufs=2, space="PSUM") as etp, \
             tc.tile_pool(name="at1p", bufs=n_chunks) as at1p, \
             tc.tile_pool(name="atp", bufs=2, space="PSUM") as atp:
            at1 = []

            def emit_at1(ch):
                tpp = etp.tile([128, 128], F32, tag="etp")
                nc.tensor.transpose(tpp[:], Av[:, ch * 128:(ch + 1) * 128],
                                    id_t[:])
                a1 = at1p.tile([128, 128], BF, tag="at1")
                nc.scalar.activation(a1[:], tpp[:], AF.Copy)
                at1.append(a1)

            cov = 0
            for d in range(G):
                o, l = seg_bounds[d]
                # alpha = exp * 1/den: ACT's per-partition scale fits rden
                nc.scalar.activation(Av[:, o:o + l], expS[:, o:o + l],
                                     AF.Copy, scale=rden_t[:, d:d + 1])
                while cov < n_chunks and (cov + 1) * 128 <= o + l:
                    emit_at1(cov)
                    cov += 1
            while cov < n_chunks:
                emit_at1(cov)
                cov += 1
            cur_ps = None
            for i, (d, ch, st, sp) in enumerate(pieces):
                if st:
                    cur_ps = atp.tile([64, 128], F32, tag="atps")
                nc.tensor.matmul(cur_ps[:], ohs_t[:, i, :], at1[ch][:],
                                 start=st, stop=sp)
                if sp:
                    nc.scalar.activation(ATs[:, d, :], cur_ps[:], AF.Copy)

        # ---- Stage agg: out[c', d] per b, heads accumulated in PSUM ----
        # Messages come straight from the bf16 XLR left half (bias included).
        with tc.tile_pool(name="gps", bufs=4, space="PSUM") as gps:
            for b in range(BC):
                gp = gps.tile([128, G], F32, tag="gp")
                xt, xo = XLR(b)
                for h in range(H):
                    nc.tensor.matmul(
                        gp[:],
                        xt[0:64, xo + h * C:xo + (h + 1) * C],
                        ATs[:, :, b * H + h],
                        start=(h == 0), stop=(h == H - 1))
                nc.scalar.activation(M1[:, b, :], gp[:], AF.Identity,
                                     bias=gb_t[:, 0:1], scale=0.25)

        # ---- Stage F: MLP ----
        with tc.tile_pool(name="fps", bufs=1, space="PSUM") as fps, \
             tc.tile_pool(name="fp", bufs=1) as fp:
            h1ps = fps.tile([BC, HID1], F32, tag="h1ps")
            for d in range(G):
                nc.tensor.matmul(h1ps[:], M1[:, :, d], W1_t[:, d, :],
                                 start=(d == 0), stop=False)
            nc.tensor.matmul(h1ps[:], ones_t[:, :BC], b1_t[:],
                             start=False, stop=True)
            h1 = fp.tile([BC, HID1], F32, tag="h1")
            nc.scalar.activation(h1[:], h1ps[:], AF.Relu)
            h1T = fp.tile([128, 4, BC], BF, tag="h1T")
            for k in range(4):
                tp = fps.tile([128, BC], F32, tag="ftp")
                nc.tensor.transpose(tp[:], h1[:, k * 128:(k + 1) * 128],
                                    id_t[0:BC, 0:BC])
                nc.scalar.activation(h1T[:, k, :], tp[:], AF.Copy)
            h2ps = fps.tile([BC, HID2], F32, tag="h2ps")
            for k in range(4):
                nc.tensor.matmul(h2ps[:], h1T[:, k, :], W2_t[:, k, :],
                                 start=(k == 0), stop=False)
            nc.tensor.matmul(h2ps[:], ones_t[:, :BC], b2_t[:],
                             start=False, stop=True)
            h2 = fp.tile([BC, HID2], F32, tag="h2")
            nc.scalar.activation(h2[:], h2ps[:], AF.Relu)
            h2tp = fps.tile([HID2, BC], F32, tag="h2tp")
            nc.tensor.transpose(h2tp[:], h2[:], id_t[0:BC, 0:BC])
            h2T = fp.tile([HID2, BC], F32, tag="h2T")
            nc.scalar.activation(h2T[:], h2tp[:], AF.Copy)
            ops = fps.tile([BC, 1], F32, tag="ops")
            nc.tensor.matmul(ops[:], h2T[:], W3_t[:], start=True, stop=True)
            outs = fp.tile([BC, 1], F32, tag="outs")
            nc.scalar.activation(outs[:], ops[:], AF.Copy)
            nc.sync.dma_start(outd, outs[:])

    nc.compile()
    return nc


def _host_prep(inputs):
    pe = np.asarray(inputs["protein_embeddings"], np.float32)
    E = _prep_edges(np.asarray(inputs["edge_index"]))

    att = np.asarray(inputs["att"], np.float32)  # [H, C]
    attflat = att.reshape(1, HC)
    attrep = np.broadcast_to(attflat, (ZB, HC)).reshape(1, ZB * HC)
    attrep = np.broadcast_to(attrep, (128, ZB * HC)).astype(bf16)

    Win = np.asarray(inputs["W_in"], np.float32)  # [G, IN, C]
    WinK = np.ascontiguousarray(
        Win.reshape(G, KC, 128, C).transpose(1, 2, 0, 3)
        .reshape(KC, 128, G * C)).astype(bf16)
    bl = np.asarray(inputs["b_l"], np.float32).reshape(1, HC)
    br = np.asarray(inputs["b_r"], np.float32).reshape(1, HC)

    shared = {
        "WinK": WinK,
        # bias broadcast to the stage-A PSUM bank layout [c, (g, b)]
        "binb": np.ascontiguousarray(np.broadcast_to(
            np.asarray(inputs["b_in"], np.float32).T[:, :, None],
            (C, G, BC)).reshape(C, G * BC)).astype(bf16),
        "onesv": np.ones((1, 64), np.float32),
        "Wl": np.asarray(inputs["W_l"], np.float32).astype(bf16),
        "Wr": np.asarray(inputs["W_r"], np.float32).astype(bf16),
        "blT": np.ascontiguousarray(np.broadcast_to(bl, (G, HC))),
        "brT": np.ascontiguousarray(np.broadcast_to(br, (G, HC))),
        "OH": E["OH"],
        "OHseg": E["oh_seg"],
        "cnt720": np.ascontiguousarray(
            np.broadcast_to(E["cnt720"][None, :], (128, E["P"]))),
        "attrep": np.ascontiguousarray(attrep),
        "gbias": np.asarray(inputs["bias"], np.float32).reshape(C, 1),
        "ident": np.eye(128, dtype=np.float32),
        "identb": np.eye(128, dtype=np.float32).astype(bf16),
        "W1": np.asarray(inputs["W1"], np.float32).astype(bf16),
        "b1v": np.asarray(inputs["b1"], np.float32).reshape(1, HID1),
        "W2": np.asarray(inputs["W2"], np.float32).astype(bf16),
        "b2v": np.asarray(inputs["b2"], np.float32).reshape(1, HID2),
        "W3": np.ascontiguousarray(np.asarray(inputs["W3"], np.float32)),
    }
    in_maps = []
    for j in range(NCORES):
        pes = pe[:, j * BC:(j + 1) * BC, :]  # [G, BC, IN]
        peT = np.ascontiguousarray(pes.transpose(2, 0, 1)) \
            .reshape(KC, 128, G * BC).astype(bf16)
        m = dict(shared)
        m["peT"] = np.ascontiguousarray(peT)
        in_maps.append(m)
    return E, in_maps


def kernel(**inputs):
    from concourse.bass_utils import run_bass_kernel_spmd
    E, in_maps = _host_prep(inputs)
    nc = _build(E)
    res = run_bass_kernel_spmd(nc, in_maps, list(range(NCORES)))
    b3 = np.asarray(inputs["b3"], np.float32).reshape(1, 1)
    out = np.concatenate([res.results[j]["out"] for j in range(NCORES)],
                         axis=0) + b3
    return out.astype(np.float32)


# revision 68
# speedup vs baseline: 1.0349x; 1.0349x over previous
"""GATv2 gene-graph kernel for 8 Trainium2 NeuronCores (Bass/Tile).

Strategy (data-parallel over batch, per the sharding hint):
- Host: shard batch (B=256 -> 32/core), precompute edge structure as static
  one-hot matrices (edge_index is data, known at trace time).
- All PE matmuls run bf16 (inputs converted on host; PSUM accumulates fp32).
- Per-gene input linear: kc-outer PE matmuls; weights arrive as 10 large
  [128, G*C] bf16 DMAs (one per kc); per-gene bias + LeakyReLU(0.01) fused
  into the ACT evacuation (bias is per-partition = per-channel).
- GATv2 attention: deduped (dst,src) pairs; z = x_l[src]+x_r[dst] via static
  one-hot PE matmuls out of the bf16 XLR tile (b_l/b_r added to XLR by DVE
  during the stage-B evacuation, so no per-batch bias matmuls);
  LeakyReLU(0.2) on ACT; att-dot via DVE mul + multi-dim reduce; segment
  softmax without max-subtraction (logits are tiny) using a degree-6 poly
  exp with ln(edge-count) folded in to handle duplicate edges.
- Aggregation: A^T built per-destination with masked one-hot PE matmuls
  (bf16), then dense bf16 PE matmuls over heads straight out of XLR (the
  message features are never written to DRAM).
- Output MLP: bf16 PE matmuls, W1 arrives as one 8.4MB DMA early.
"""
import sys
from contextlib import ExitStack

import numpy as np

sys.path.insert(0, "/opt/trn_rl_repo")

import ml_dtypes  # noqa: E402
import concourse.bass as bass  # noqa: E402
import concourse.tile as tile  # noqa: E402
from concourse import bacc, mybir  # noqa: E402

bf16 = ml_dtypes.bfloat16
F32 = mybir.dt.float32
BF = mybir.dt.bfloat16
AF = mybir.ActivationFunctionType
ALU = mybir.AluOpType

G, B, IN, C, H = 64, 256, 1280, 128, 4
HC = H * C  # 512
KC = IN // 128  # 10
NCORES = 8
BC = B // NCORES  # 32
HID1, HID2 = 512, 128
ZB = 4  # batch elements per z-group (DVE op granularity)


def _prep_edges(edge_index):
    sl = np.arange(G, dtype=np.int64)
    src = np.concatenate([np.asarray(edge_index[0]), sl])
    dst = np.concatenate([np.asarray(edge_index[1]), sl])
    upairs, cnt = np.unique(dst * G + src, return_counts=True)
    pd = (upairs // G).astype(np.int64)
    ps = (upairs % G).astype(np.int64)
    p_real = len(upairs)
    n_chunks = (p_real + 127) // 128
    P = n_chunks * 128
    seg_len = np.bincount(pd, minlength=G)
    seg_off = np.zeros(G, np.int64)
    seg_off[1:] = np.cumsum(seg_len)[:-1]
    cnt720 = np.zeros(P, np.float32)
    cnt720[:p_real] = (cnt.astype(np.float64) / 720.0).astype(np.float32)
    ps_pad = np.zeros(P, np.int64)
    ps_pad[:p_real] = ps
    pd_pad = np.full(P, G - 1, np.int64)
    pd_pad[:p_real] = pd

    OH = np.zeros((n_chunks, 128, 128), bf16)
    for p in range(P):
        ch, k = p // 128, p % 128
        OH[ch, ps_pad[p], k] = 1
        OH[ch, G + pd_pad[p], k] = 1

    # AT-build plan: per destination d, pieces of its (real) segment per chunk,
    # as zero-masked one-hots [128(pair-in-chunk), 64(g)].
    pieces = []  # (d, ch, is_first, is_last)
    oh_seg = []
    for d in range(G):
        o, l = int(seg_off[d]), int(seg_len[d])
        if l == 0:
            continue  # cannot happen (self loops guarantee l>=1)
        ch_lo, ch_hi = o // 128, (o + l - 1) // 128
        plist = []
        for ch in range(ch_lo, ch_hi + 1):
            lo = max(o, ch * 128)
            hi = min(o + l, (ch + 1) * 128)
            m = np.zeros((128, 64), bf16)
            for p in range(lo, hi):
                m[p % 128, ps_pad[p]] = 1
            plist.append((ch, m))
        for i, (ch, m) in enumerate(plist):
            pieces.append((d, ch, i == 0, i == len(plist) - 1))
            oh_seg.append(m)
    oh_seg = np.stack(oh_seg)  # [n_pieces, 128, 64]

    # segment bounds on the padded list (pads live in d=63's tail; their
    # cnt720=0 makes them contribute 0 to the denominator)
    seg_bounds = []
    for d in range(G):
        o, l = int(seg_off[d]), int(seg_len[d])
        if d == G - 1:
            l += P - p_real
        seg_bounds.append((o, l))

    return dict(P=P, n_chunks=n_chunks, cnt720=cnt720, OH=OH,
                oh_seg=oh_seg, pieces=pieces, seg_bounds=seg_bounds)


def _build(E, lrelu_act=True, poly_exp=True):
    P, n_chunks = E["P"], E["n_chunks"]
    pieces = E["pieces"]
    n_pieces = len(pieces)

    nc = bacc.Bacc("TRN2", target_bir_lowering=False, debug=False)

    def din(name, shape, dt=F32):
        return nc.dram_tensor(name, list(shape), dt, kind="ExternalInput").ap()

    peT = din("peT", [KC, 128, G * BC], BF)
    WinK = din("WinK", [KC, 128, G * C], BF)
    binb = din("binb", [128, G * BC], BF)
    onesv = din("onesv", [1, 64])
    Wl = din("Wl", [C, HC], BF)
    Wr = din("Wr", [C, HC], BF)
    blT = din("blT", [G, HC])
    brT = din("brT", [G, HC])
    OHd = din("OH", [n_chunks, 128, 128], BF)
    OHseg = din("OHseg", [n_pieces, 128, 64], BF)
    cntd = din("cnt720", [128, P])
    attrep = din("attrep", [128, ZB * HC], BF)
    gbias = din("gbias", [C, 1])
    identd = din("ident", [128, 128])
    identb = din("identb", [128, 128], BF)
    W1d = din("W1", [G * C, HID1], BF)
    b1v = din("b1v", [1, HID1])
    W2d = din("W2", [HID1, HID2], BF)
    b2v = din("b2v", [1, HID2])
    W3d = din("W3", [HID2, 1])
    outd = nc.dram_tensor("out", [BC, 1], F32, kind="ExternalOutput").ap()

    with tile.TileContext(nc) as tc, ExitStack() as ctx:
        pers = ctx.enter_context(tc.tile_pool(name="pers", bufs=1))

        # persistent data tiles
        xT = pers.tile([128, G, BC], BF, tag="xT")
        # one tile per 4-batch group: tile-granular dependency tracking
        # would otherwise serialize all of stage C behind all of stage B
        XLRg = [pers.tile([128, ZB * HC], BF, tag=f"XLR{g}", name=f"XLR{g}")
                for g in range(BC // ZB)]

        def XLR(b):
            return XLRg[b // ZB], (b % ZB) * HC
        Sv = pers.tile([128, P], F32, tag="Sv")
        expS = pers.tile([128, P], F32, tag="expS")
        Av = expS  # alpha overwrites the exp values in place
        pt_ = pers.tile([128, P], F32, tag="polyt")
        ATs = pers.tile([64, G, 128], BF, tag="ATs")
        M1 = pers.tile([128, BC, G], BF, tag="M1")

        # constants (tiles declared here; DMAs issued after stage A's first
        # weight tiles so they don't delay the critical path)
        ones_t = pers.tile([1, 64], F32, tag="ones")
        binb_t = pers.tile([128, G * BC], BF, tag="binb")
        Wl_t = pers.tile([C, HC], BF, tag="Wl")
        Wr_t = pers.tile([C, HC], BF, tag="Wr")
        blT_t = pers.tile([G, HC], F32, tag="blT")
        brT_t = pers.tile([G, HC], F32, tag="brT")
        lnc_t = pers.tile([128, P], F32, tag="lnc")
        att_t = pers.tile([128, ZB * 4, C], BF, tag="att")
        gb_t = pers.tile([C, 1], F32, tag="gb")
        id_t = pers.tile([128, 128], F32, tag="id")
        id_t_bf = pers.tile([128, 128], BF, tag="idb")
        b1_t = pers.tile([1, HID1], F32, tag="b1")
        b2_t = pers.tile([1, HID2], F32, tag="b2")
        W3_t = pers.tile([HID2, 1], F32, tag="W3")
        oh_t = pers.tile([128, n_chunks, 128], BF, tag="ohall")
        ohs_t = pers.tile([128, n_pieces, 64], BF, tag="ohseg")
        W1_t = pers.tile([128, G, HID1], BF, tag="W1")
        W2_t = pers.tile([128, 4, HID2], BF, tag="W2")

        def emit_consts():
            nc.sync.dma_start(ones_t[:], onesv)
            nc.sync.dma_start(binb_t[:], binb)
            nc.sync.dma_start(Wl_t[:], Wl)
            nc.sync.dma_start(Wr_t[:], Wr)
            nc.sync.dma_start(blT_t[:], blT)
            nc.sync.dma_start(brT_t[:], brT)
            nc.sync.dma_start(att_t[:],
                              attrep.rearrange("p (g c) -> p g c", c=C))
            nc.sync.dma_start(id_t[:], identd)
            nc.sync.dma_start(id_t_bf[:], identb)
            nc.sync.dma_start(oh_t[:], OHd.rearrange("n p q -> p n q"))
            nc.sync.dma_start(lnc_t[:], cntd)
            nc.sync.dma_start(gb_t[:], gbias)
            nc.sync.dma_start(b1_t[:], b1v)
            nc.sync.dma_start(b2_t[:], b2v)
            nc.sync.dma_start(W3_t[:], W3d)

        def emit_lrelu(out_ap, in_ap, alpha, eng=None):
            if lrelu_act:
                nc.scalar.activation(out_ap, in_ap, AF.Lrelu, alpha=alpha)
            else:
                (eng or nc.vector).scalar_tensor_tensor(
                    out_ap, in_ap, alpha, in_ap, ALU.mult, ALU.max)

        # ---- Stage A: per-gene input linear ----
        # kc-outer; all 64 genes accumulate in 4 persistent PSUM banks
        # (bank q holds genes 16q..16q+15 as [128c, 16*32b]). Weights arrive
        # as one [128, G*C] bf16 DMA per kc; per-gene bias + LeakyReLU(0.01)
        # fused into the evacuation.
        with tc.tile_pool(name="pep", bufs=2) as pep, \
             tc.tile_pool(name="wkp", bufs=2) as wkp, \
             tc.tile_pool(name="aps", bufs=1, space="PSUM") as aps:
            xbank = [aps.tile([128, 512], F32, tag=f"xb{q}", name=f"xb{q}")
                     for q in range(4)]
            for kc in range(KC):
                wk = wkp.tile([128, G * C], BF, tag="wk")
                nc.sync.dma_start(wk[:], WinK[kc])
                pt = pep.tile([128, G * BC], BF, tag="pe")
                nc.sync.dma_start(pt[:], peT[kc])
                if kc == 0:
                    emit_consts()
                for g in range(G):
                    nc.tensor.matmul(
                        xbank[g // 16][:, (g % 16) * BC:(g % 16 + 1) * BC],
                        wk[:, g * C:(g + 1) * C], pt[:, g * BC:(g + 1) * BC],
                        start=(kc == 0 and g % 16 == 0),
                        stop=(kc == KC - 1 and g % 16 == 15))
            # bias added into PSUM by DVE (idle in this window), so the
            # evacuation is 4 wide bank-sized lrelus instead of 64 per-gene
            for q in range(4):
                nc.vector.tensor_add(xbank[q][:], xbank[q][:],
                                     binb_t[:, q * 512:(q + 1) * 512])
                emit_lrelu(xT[:, q * 16:(q + 1) * 16, :], xbank[q][:], 0.01)

        # ---- gated prefetch of big late-stage tensors ----
        # A 1-element copy from xT makes these DMAs wait for stage A's end,
        # keeping the DMA engines clear for the critical wk/pt stream; they
        # then overlap stages B..E (the data isn't needed until E/F).
        nc.vector.tensor_copy(ohs_t[0:1, 0:1, 0:1], xT[0:1, 0:1, 0:1])
        nc.vector.tensor_copy(W1_t[0:1, 0:1, 0:1], xT[0:1, 0:1, 0:1])
        nc.sync.dma_start(ohs_t[:], OHseg.rearrange("n p g -> p n g"))
        nc.sync.dma_start(W1_t[:], W1d.rearrange("(d p) h -> p d h", p=128))
        nc.sync.dma_start(W2_t[:], W2d.rearrange("(k p) c -> p k c", p=128))

        # ---- Stage B: x_l / x_r transforms ----
        # Biases b_l/b_r ride into XLR via the DVE evacuation adds.
        with tc.tile_pool(name="bps", bufs=4, space="PSUM") as bps:
            for b in range(BC):
                xsl = xT[:, :, b]  # [128c, 64g]
                psl = bps.tile([64, HC], F32, tag="psl")
                nc.tensor.matmul(psl[:], xsl, Wl_t[:], start=True, stop=True)
                xt, xo = XLR(b)
                nc.vector.tensor_add(xt[0:64, xo:xo + HC], psl[:], blT_t[:])
                psr = bps.tile([64, HC], F32, tag="psr")
                nc.tensor.matmul(psr[:], xsl, Wr_t[:], start=True, stop=True)
                nc.vector.tensor_add(xt[64:128, xo:xo + HC], psr[:], brT_t[:])

        # ---- Stage C: pair features + scores ----
        den_t = pers.tile([128, 64], F32, tag="den")
        rden_t = pers.tile([128, 64], F32, tag="rden")
        with tc.tile_pool(name="zps", bufs=3, space="PSUM") as zps, \
             tc.tile_pool(name="zlp", bufs=3) as zlp, \
             tc.tile_pool(name="sap", bufs=2) as sap, \
             tc.tile_pool(name="tps", bufs=2, space="PSUM") as tps:
            seg_bounds = E["seg_bounds"]
            ends_in_ch = [[] for _ in range(n_chunks)]
            for dd in range(G):
                o, l = seg_bounds[dd]
                ends_in_ch[(o + l - 1) // 128].append(dd)

            def emit_score_tail(sat_, ch_):
                # transpose + poly-exp + this chunk's finished segment sums;
                # emitted one chunk late so the PE's in-order queue never
                # stalls on the DVE reduce feeding the transpose
                tp = tps.tile([128, 128], BF, tag="tp")
                nc.tensor.transpose(tp[:], sat_[:], id_t_bf[:])
                sv = Sv[:, ch_ * 128:(ch_ + 1) * 128]
                nc.scalar.activation(sv, tp[:], AF.Copy)
                if poly_exp:
                    # 720*exp(x) ~= (((((x+6)x+30)x+120)x+360)x+720)x+720
                    # then alpha_unnorm = (t+720) * cnt/720 (cnt=0 => pad)
                    pc_ = pt_[:, ch_ * 128:(ch_ + 1) * 128]
                    ec = expS[:, ch_ * 128:(ch_ + 1) * 128]
                    lc = lnc_t[:, ch_ * 128:(ch_ + 1) * 128]
                    nc.vector.scalar_tensor_tensor(pc_, sv, 6.0, sv,
                                                   ALU.add, ALU.mult)
                    for c in (30.0, 120.0, 360.0, 720.0):
                        nc.vector.scalar_tensor_tensor(pc_, pc_, c, sv,
                                                       ALU.add, ALU.mult)
                    nc.vector.scalar_tensor_tensor(ec, pc_, 720.0, lc,
                                                   ALU.add, ALU.mult)

            pending = None
            for ch in range(n_chunks):
                sat = sap.tile([128, 128], BF, tag="sa")
                for bg in range(BC // ZB):
                    if bg == 5 and pending is not None:
                        # previous chunk's tail, five groups late: the DVE
                        # runs ~1us/group behind the PE, so this is when its
                        # sat is actually complete and the PE's in-order
                        # queue won't stall on the transpose
                        emit_score_tail(*pending)
                        pending = None
                    zt = zlp.tile([128, ZB * 4, C], BF, tag="zt")
                    for half in range(2):
                        # two matmuls into adjacent PSUM banks, one wide
                        # evacuation (halves the ACT instruction count)
                        zp = zps.tile([128, 2 * HC], F32, tag="zp")
                        for bi2 in range(2):
                            b = bg * ZB + half * 2 + bi2
                            xt, xo = XLR(b)
                            nc.tensor.matmul(
                                zp[:, bi2 * HC:(bi2 + 1) * HC],
                                oh_t[:, ch, :],
                                xt[:, xo:xo + HC],
                                start=True, stop=True)
                        emit_lrelu(zt[:, half * 8:(half + 1) * 8, :],
                                   zp[:], 0.2)
                    nc.vector.tensor_mul(zt[:], zt[:], att_t[:])
                    # pairwise in-place folds shrink the (slow, 1x-rate)
                    # reduce read from 2048 to 512 elements per lane
                    nc.vector.tensor_add(zt[:, :, 0:64], zt[:, :, 0:64],
                                         zt[:, :, 64:128])
                    nc.vector.tensor_add(zt[:, :, 0:32], zt[:, :, 0:32],
                                         zt[:, :, 32:64])
                    with nc.allow_low_precision(reason="score sums are tiny"):
                        nc.vector.tensor_reduce(
                            sat[:, bg * ZB * 4:(bg + 1) * ZB * 4],
                            zt[:, :, 0:32],
                            axis=mybir.AxisListType.X, op=ALU.add)
                pending = (sat, ch)
            emit_score_tail(*pending)

        # ---- Stage D: softmax normalization ----
        if not poly_exp:
            nc.scalar.activation(expS[:], Sv[:], AF.Exp)
            nc.vector.tensor_mul(expS[:], expS[:], lnc_t[:])
        for d in range(G):
            o, l = seg_bounds[d]
            nc.vector.tensor_reduce(den_t[:, d:d + 1], expS[:, o:o + l],
                                    axis=mybir.AxisListType.X, op=ALU.add)
        nc.vector.reciprocal(rden_t[:], den_t[:])

        # ---- Stage E: build AT[g, d, bh] (bf16), pipelined with the
        # alpha normalization: each chunk's transpose is emitted as soon
        # as the segments covering its columns are scaled ----
        with tc.tile_pool(name="etp", bufs=2, space="PSUM") as etp, \
             tc.tile_pool(name="at1p", bufs=n_chunks) as at1p, \
             tc.tile_pool(name="atp", bufs=2, space="PSUM") as atp:
            at1 = []

            def emit_at1(ch):
                tpp = etp.tile([128, 128], F32, tag="etp")
                nc.tensor.transpose(tpp[:], Av[:, ch * 128:(ch + 1) * 128],
                                    id_t[:])
                a1 = at1p.tile([128, 128], BF, tag="at1")
                nc.scalar.activation(a1[:], tpp[:], AF.Copy)
                at1.append(a1)

            cov = 0
            for d in range(G):
                o, l = seg_bounds[d]
                # alpha = exp * 1/den on DVE: the ACT queue must stay clear
                # for the at1 copies that feed stage E's piece matmuls
                nc.vector.tensor_scalar_mul(Av[:, o:o + l], expS[:, o:o + l],
                                            rden_t[:, d:d + 1])
                while cov < n_chunks and (cov + 1) * 128 <= o + l:
                    emit_at1(cov)
                    cov += 1
            while cov < n_chunks:
                emit_at1(cov)
                cov += 1
            cur_ps = None
            for i, (d, ch, st, sp) in enumerate(pieces):
                if st:
                    cur_ps = atp.tile([64, 128], F32, tag="atps")
                nc.tensor.matmul(cur_ps[:], ohs_t[:, i, :], at1[ch][:],
                                 start=st, stop=sp)
                if sp:
                    nc.scalar.activation(ATs[:, d, :], cur_ps[:], AF.Copy)

        # ---- Stage agg: out[c', d] per b, heads accumulated in PSUM ----
        # Messages come straight from the bf16 XLR left half (bias included).
        with tc.tile_pool(name="gps", bufs=4, space="PSUM") as gps:
            for b in range(BC):
                gp = gps.tile([128, G], F32, tag="gp")
                xt, xo = XLR(b)
                for h in range(H):
                    nc.tensor.matmul(
                        gp[:],
                        xt[0:64, xo + h * C:xo + (h + 1) * C],
                        ATs[:, :, b * H + h],
                        start=(h == 0), stop=(h == H - 1))
                nc.scalar.activation(M1[:, b, :], gp[:], AF.Identity,
                                     bias=gb_t[:, 0:1], scale=0.25)

        # ---- Stage F: MLP ----
        with tc.tile_pool(name="fps", bufs=1, space="PSUM") as fps, \
             tc.tile_pool(name="fp", bufs=1) as fp:
            h1ps = fps.tile([BC, HID1], F32, tag="h1ps")
            for d in range(G):
                nc.tensor.matmul(h1ps[:], M1[:, :, d], W1_t[:, d, :],
                                 start=(d == 0), stop=False)
            nc.tensor.matmul(h1ps[:], ones_t[:, :BC], b1_t[:],
                             start=False, stop=True)
            h1 = fp.tile([BC, HID1], F32, tag="h1")
            nc.scalar.activation(h1[:], h1ps[:], AF.Relu)
            h1T = fp.tile([128, 4, BC], BF, tag="h1T")
            for k in range(4):
                tp = fps.tile([128, BC], F32, tag="ftp")
                nc.tensor.transpose(tp[:], h1[:, k * 128:(k + 1) * 128],
                                    id_t[0:BC, 0:BC])
                nc.scalar.activation(h1T[:, k, :], tp[:], AF.Copy)
            h2ps = fps.tile([BC, HID2], F32, tag="h2ps")
            for k in range(4):
                nc.tensor.matmul(h2ps[:], h1T[:, k, :], W2_t[:, k, :],
                                 start=(k == 0), stop=False)
            nc.tensor.matmul(h2ps[:], ones_t[:, :BC], b2_t[:],
                             start=False, stop=True)
            h2 = fp.tile([BC, HID2], F32, tag="h2")
            nc.scalar.activation(h2[:], h2ps[:], AF.Relu)
            h2tp = fps.tile([HID2, BC], F32, tag="h2tp")
            nc.tensor.transpose(h2tp[:], h2[:], id_t[0:BC, 0:BC])
            h2T = fp.tile([HID2, BC], F32, tag="h2T")
            nc.scalar.activation(h2T[:], h2tp[:], AF.Copy)
            ops = fps.tile([BC, 1], F32, tag="ops")
            nc.tensor.matmul(ops[:], h2T[:], W3_t[:], start=True, stop=True)
            outs = fp.tile([BC, 1], F32, tag="outs")
            nc.scalar.activation(outs[:], ops[:], AF.Copy)
            nc.sync.dma_start(outd, outs[:])

    nc.compile()
    return nc


def _host_prep(inputs):
    pe = np.asarray(inputs["protein_embeddings"], np.float32)
    E = _prep_edges(np.asarray(inputs["edge_index"]))

    att = np.asarray(inputs["att"], np.float32)  # [H, C]
    attflat = att.reshape(1, HC)
    attrep = np.broadcast_to(attflat, (ZB, HC)).reshape(1, ZB * HC)
    attrep = np.broadcast_to(attrep, (128, ZB * HC)).astype(bf16)

    Win = np.asarray(inputs["W_in"], np.float32)  # [G, IN, C]
    WinK = np.ascontiguousarray(
        Win.reshape(G, KC, 128, C).transpose(1, 2, 0, 3)
        .reshape(KC, 128, G * C)).astype(bf16)
    bl = np.asarray(inputs["b_l"], np.float32).reshape(1, HC)
    br = np.asarray(inputs["b_r"], np.float32).reshape(1, HC)

    shared = {
        "WinK": WinK,
        # bias broadcast to the stage-A PSUM bank layout [c, (g, b)]
        "binb": np.ascontiguousarray(np.broadcast_to(
            np.asarray(inputs["b_in"], np.float32).T[:, :, None],
            (C, G, BC)).reshape(C, G * BC)).astype(bf16),
        "onesv": np.ones((1, 64), np.float32),
        "Wl": np.asarray(inputs["W_l"], np.float32).astype(bf16),
        "Wr": np.asarray(inputs["W_r"], np.float32).astype(bf16),
        "blT": np.ascontiguousarray(np.broadcast_to(bl, (G, HC))),
        "brT": np.ascontiguousarray(np.broadcast_to(br, (G, HC))),
        "OH": E["OH"],
        "OHseg": E["oh_seg"],
        "cnt720": np.ascontiguousarray(
            np.broadcast_to(E["cnt720"][None, :], (128, E["P"]))),
        "attrep": np.ascontiguousarray(attrep),
        "gbias": np.asarray(inputs["bias"], np.float32).reshape(C, 1),
        "ident": np.eye(128, dtype=np.float32),
        "identb": np.eye(128, dtype=np.float32).astype(bf16),
        "W1": np.asarray(inputs["W1"], np.float32).astype(bf16),
        "b1v": np.asarray(inputs["b1"], np.float32).reshape(1, HID1),
        "W2": np.asarray(inputs["W2"], np.float32).astype(bf16),
        "b2v": np.asarray(inputs["b2"], np.float32).reshape(1, HID2),
        "W3": np.ascontiguousarray(np.asarray(inputs["W3"], np.float32)),
    }
    in_maps = []
    for j in range(NCORES):
        pes = pe[:, j * BC:(j + 1) * BC, :]  # [G, BC, IN]
        peT = np.ascontiguousarray(pes.transpose(2, 0, 1)) \
            .reshape(KC, 128, G * BC).astype(bf16)
        m = dict(shared)
        m["peT"] = np.ascontiguousarray(peT)
        in_maps.append(m)
    return E, in_maps


def kernel(**inputs):
    from concourse.bass_utils import run_bass_kernel_spmd
    E, in_maps = _host_prep(inputs)
    nc = _build(E)
    res = run_bass_kernel_spmd(nc, in_maps, list(range(NCORES)))
    b3 = np.asarray(inputs["b3"], np.float32).reshape(1, 1)
    out = np.concatenate([res.results[j]["out"] for j in range(NCORES)],
                         axis=0) + b3
    return out.astype(np.float32)
